# revision 1
# baseline (speedup 1.0000x reference)
"""Gaussian 2x2 splat (DifferentiableSquareSensor) on 8 Trainium2 NeuronCores.

Full inputs in, full 1024x1024 image out.

Math: x,y are uniform in [0,1), so pixel coords land in [512,1024) and with
sigma=0.1 every Gaussian tap except the nearest 2x2 neighborhood is <= e^-50
(~2e-22 relative) -- invisible in fp32.  The splat therefore reduces to a
separable 2x2 deposit with weights  g(t)=exp(-50 t^2), g(1-t)  per axis,
normalized by (gx0+gx1)(gy0+gy1).

Distribution: points are sharded to cores by 64-column x-strip of the active
512x512 region (with boundary-column duplication), and within a core are
bucketed by 32-row y-band (with boundary-row duplication).  Each core
computes its [512, 64] strip on-device:
  phase A: bulk fp32 coordinate/weight math (ACT + DVE)
  phase B: per-128-point-block one-hot placement tiles built with
           broadcast-AP tensor ops, then two PE matmuls per block
           accumulate the 2x2 outer products into PSUM band accumulators.
The host only shards/buckets/pads inputs and reassembles the strips.
"""

import json
import os
import sys

import numpy as np

for _p in ("/opt/trn_rl_repo", "/root/.axon_site/_ro/trn_rl_repo"):
    if os.path.isdir(_p) and _p not in sys.path:
        sys.path.append(_p)

import concourse.bass as bass
import concourse.mybir as mybir
from concourse.bass_utils import run_bass_kernel_spmd
from concourse.tile import TileContext

P = 128
NCORES = 8
STRIP_W = 64          # columns per core
BAND_H = 32           # rows per y-band
NBANDS = 512 // BAND_H  # 16
NBATCH = 66           # max blocks per batched build group (SBUF-sized)
F32 = mybir.dt.float32
F16 = mybir.dt.float16


def _split_multiwait(nc):
    """This walrus build rejects >1 sync-wait per instruction; split extras
    into single-wait NoOps placed immediately before on the same engine."""
    orig = nc.to_json_bytes

    def patched():
        js = json.loads(orig().decode())
        for fn in js["functions"]:
            for blk in fn["blocks"]:
                newlist = []
                for inst in blk["instructions"]:
                    si = inst.get("sync_info")
                    ow = (si or {}).get("on_wait") or []
                    if len(ow) > 1:
                        for k, w in enumerate(ow[:-1]):
                            newlist.append({
                                "name": f"{inst['name']}-w{k}",
                                "opcode": "NoOp",
                                "engine": inst["engine"],
                                "ins": [], "outs": [],
                                "sync_info": {"on_wait": [w], "on_update": []},
                                "bass_nofuse": True,
                            })
                        si["on_wait"] = [ow[-1]]
                    newlist.append(inst)
                blk["instructions"] = newlist
        return json.dumps(js).encode()

    nc.to_json_bytes = patched


def _floor_frac(nc, pool, XP, C):
    """Exact floor/frac of XP (values in [512,1024)) via round-to-nearest
    int conversion of XP-0.5 plus a frac==1.0 fixup (round-half-even at
    integer inputs)."""
    XI = pool.tile([P, C], mybir.dt.int32, name="fl_i")
    nc.vector.tensor_scalar(out=XI[:], in0=XP[:], scalar1=0.5, scalar2=None,
                            op0=mybir.AluOpType.subtract)
    XIF = pool.tile([P, C], F32, name="fl_f")
    nc.vector.tensor_copy(XIF[:], XI[:])
    FR = pool.tile([P, C], F32, name="fl_fr")
    nc.vector.tensor_tensor(out=FR[:], in0=XP[:], in1=XIF[:],
                            op=mybir.AluOpType.subtract)
    FIX = pool.tile([P, C], F32, name="fl_fix")
    nc.vector.tensor_scalar(out=FIX[:], in0=FR[:], scalar1=1.0, scalar2=None,
                            op0=mybir.AluOpType.is_ge)
    nc.vector.tensor_tensor(out=FR[:], in0=FR[:], in1=FIX[:],
                            op=mybir.AluOpType.subtract)
    nc.vector.tensor_tensor(out=XIF[:], in0=XIF[:], in1=FIX[:],
                            op=mybir.AluOpType.add)
    return XIF, FR


def _build_module(nbb):
    """Build the SPMD bass module for per-band block-column count nbb.
    Total columns NB = 16*nbb; batches within a band may have a partial
    tail."""
    NB = NBANDS * nbb
    nc = bass.Bass("TRN2", target_bir_lowering=False, debug=False,
                   num_devices=NCORES)
    xs_d = nc.dram_tensor("xs", [P, NB], F32, kind="ExternalInput")
    ys_d = nc.dram_tensor("ys", [P, NB], F32, kind="ExternalInput")
    vs_d = nc.dram_tensor("vs", [P, NB], F32, kind="ExternalInput")
    strip_d = nc.dram_tensor("strip", [512, STRIP_W], F32,
                             kind="ExternalOutput")

    CA = 512  # phase-A chunk columns
    nchunks = (NB + CA - 1) // CA
    # split each band into as few even batches as SBUF allows: ceil-even of
    # nbb/k for the smallest k with result <= NBATCH (nbb=130 -> 66+64)
    nbatch = nbb
    k = 1
    while nbatch > NBATCH:
        k += 1
        nbatch = -(-nbb // k)
        nbatch += nbatch % 2

    with TileContext(nc) as tc:
        with (
            tc.tile_pool(name="persist", bufs=1) as pers,
            tc.tile_pool(name="chunk", bufs=2) as chk,
            tc.tile_pool(name="ftmp", bufs=1) as ftmp,
            tc.tile_pool(name="batch", bufs=2) as bat,
            tc.tile_pool(name="psum", bufs=1, space="PSUM") as psp,
        ):
            # ---- one-time constants ----
            PIDU = pers.tile([P, 1], mybir.dt.uint32)
            nc.gpsimd.dma_start(
                PIDU[:], nc.partition_id_tensor[0:1, 0:1].to_broadcast([P, 1]))
            PIDF = pers.tile([P, 1], F32)
            nc.vector.tensor_copy(PIDF[:], PIDU[:])
            # SCX = 511 + 64*pid  (cxp1 = xb - SCX)
            SCX = pers.tile([P, 1], F32)
            nc.vector.tensor_scalar(out=SCX[:], in0=PIDF[:], scalar1=64.0,
                                    scalar2=511.0, op0=mybir.AluOpType.mult,
                                    op1=mybir.AluOpType.add)
            # RB[p, j] = 511 + 32*(j // nbb)   (ryc = yb - RB)
            RB = pers.tile([P, NB], F32)
            nc.gpsimd.iota(RB[:], pattern=[[BAND_H, NBANDS], [0, nbb]],
                           base=511, channel_multiplier=0,
                           allow_small_or_imprecise_dtypes=True)
            # pair-duplicated iotas: values 0,0,1,1,... so two blocks'
            # one-hots interleave in adjacent fp16 lanes (DVE 2x mode)
            XIOTA = pers.tile([P, 132], F16)
            nc.gpsimd.iota(XIOTA[:], pattern=[[1, 66], [0, 2]], base=0,
                           channel_multiplier=0,
                           allow_small_or_imprecise_dtypes=True)
            YIOTA = pers.tile([P, 68], F16)
            nc.gpsimd.iota(YIOTA[:], pattern=[[1, 34], [0, 2]], base=0,
                           channel_multiplier=0,
                           allow_small_or_imprecise_dtypes=True)

            # ---- per-point arrays, one tile per phase-A chunk so that
            # phase-B batches only depend on their own chunk (overlap) ----
            def chunk_tiles(nm):
                return [pers.tile([P, min(CA, NB - i * CA)], F16,
                                  name=f"{nm}{i}") for i in range(nchunks)]
            CXP1s = chunk_tiles("CXP1")
            RYCs = chunk_tiles("RYC")
            GY0s = chunk_tiles("GY0")
            GY1s = chunk_tiles("GY1")
            A0s = chunk_tiles("A0")
            A1s = chunk_tiles("A1")

            # ---- phase A ----
            for ci in range(nchunks):
                j0 = ci * CA
                C = min(CA, NB - j0)
                sl = slice(j0, j0 + C)
                X = chk.tile([P, CA], F32, name="X")
                Y = chk.tile([P, CA], F32, name="Y")
                V = chk.tile([P, CA], F32, name="V")
                nc.sync.dma_start(X[:, :C], xs_d[:, sl])
                nc.sync.dma_start(Y[:, :C], ys_d[:, sl])
                nc.sync.dma_start(V[:, :C], vs_d[:, sl])

                XP = ftmp.tile([P, CA], F32, name="XP")
                nc.scalar.activation(XP[:, :C], X[:, :C],
                                     mybir.ActivationFunctionType.Copy,
                                     bias=512.0, scale=512.0)
                YP = ftmp.tile([P, CA], F32, name="YP")
                nc.scalar.activation(YP[:, :C], Y[:, :C],
                                     mybir.ActivationFunctionType.Copy,
                                     bias=512.0, scale=512.0)

                XBF, TX = _floor_frac(nc, ftmp, XP[:, :C], C)
                YBF, TY = _floor_frac(nc, ftmp, YP[:, :C], C)

                # gx0 = exp(-50 tx^2), gx1 = exp(-50 (tx-1)^2)
                SX = ftmp.tile([P, CA], F32, name="SX")
                nc.vector.tensor_tensor(out=SX[:, :C], in0=TX[:], in1=TX[:],
                                        op=mybir.AluOpType.mult)
                GX0 = ftmp.tile([P, CA], F32, name="GX0")
                nc.scalar.activation(GX0[:, :C], SX[:, :C],
                                     mybir.ActivationFunctionType.Exp,
                                     bias=0.0, scale=-50.0)
                UX = ftmp.tile([P, CA], F32, name="UX")
                nc.vector.tensor_scalar(out=UX[:, :C], in0=TX[:], scalar1=1.0,
                                        scalar2=None,
                                        op0=mybir.AluOpType.subtract)
                SUX = ftmp.tile([P, CA], F32, name="SUX")
                nc.gpsimd.tensor_tensor(out=SUX[:, :C], in0=UX[:, :C],
                                        in1=UX[:, :C],
                                        op=mybir.AluOpType.mult)
                GX1 = ftmp.tile([P, CA], F32, name="GX1")
                nc.scalar.activation(GX1[:, :C], SUX[:, :C],
                                     mybir.ActivationFunctionType.Exp,
                                     bias=0.0, scale=-50.0)

                SY = ftmp.tile([P, CA], F32, name="SY")
                nc.vector.tensor_tensor(out=SY[:, :C], in0=TY[:], in1=TY[:],
                                        op=mybir.AluOpType.mult)
                GY0F = ftmp.tile([P, CA], F32, name="GY0F")
                nc.scalar.activation(GY0F[:, :C], SY[:, :C],
                                     mybir.ActivationFunctionType.Exp,
                                     bias=0.0, scale=-50.0)
                UY = ftmp.tile([P, CA], F32, name="UY")
                nc.vector.tensor_scalar(out=UY[:, :C], in0=TY[:], scalar1=1.0,
                                        scalar2=None,
                                        op0=mybir.AluOpType.subtract)
                SUY = ftmp.tile([P, CA], F32, name="SUY")
                nc.gpsimd.tensor_tensor(out=SUY[:, :C], in0=UY[:, :C],
                                        in1=UY[:, :C],
                                        op=mybir.AluOpType.mult)
                GY1F = ftmp.tile([P, CA], F32, name="GY1F")
                nc.scalar.activation(GY1F[:, :C], SUY[:, :C],
                                     mybir.ActivationFunctionType.Exp,
                                     bias=0.0, scale=-50.0)

                # R = v / ((gx0+gx1)(gy0+gy1))
                ZX = ftmp.tile([P, CA], F32, name="ZX")
                nc.vector.tensor_tensor(out=ZX[:, :C], in0=GX0[:, :C],
                                        in1=GX1[:, :C],
                                        op=mybir.AluOpType.add)
                ZY = ftmp.tile([P, CA], F32, name="ZY")
                nc.vector.tensor_tensor(out=ZY[:, :C], in0=GY0F[:, :C],
                                        in1=GY1F[:, :C],
                                        op=mybir.AluOpType.add)
                Z = ftmp.tile([P, CA], F32, name="Z")
                nc.vector.tensor_tensor(out=Z[:, :C], in0=ZX[:, :C],
                                        in1=ZY[:, :C],
                                        op=mybir.AluOpType.mult)
                # 1/Z via exp(-ln Z) on ACT (keeps DVE free; ~1e-4 rel)
                LNZ = ftmp.tile([P, CA], F32, name="LNZ")
                nc.scalar.activation(LNZ[:, :C], Z[:, :C],
                                     mybir.ActivationFunctionType.Ln)
                RZ = ftmp.tile([P, CA], F32, name="RZ")
                nc.scalar.activation(RZ[:, :C], LNZ[:, :C],
                                     mybir.ActivationFunctionType.Exp,
                                     bias=0.0, scale=-1.0)
                # keep every fp16-stored factor bounded: gy' = gy/Zy <= 1,
                # a' = v*gx/Zx <= |v|  (their product is the exact deposit)
                TY1 = ftmp.tile([P, CA], F32, name="TY1")
                nc.vector.tensor_tensor(out=TY1[:, :C], in0=ZX[:, :C],
                                        in1=RZ[:, :C],
                                        op=mybir.AluOpType.mult)
                TX1 = ftmp.tile([P, CA], F32, name="TX1")
                nc.gpsimd.tensor_tensor(out=TX1[:, :C], in0=ZY[:, :C],
                                        in1=RZ[:, :C],
                                        op=mybir.AluOpType.mult)
                nc.vector.tensor_tensor(out=GY0s[ci][:, :C], in0=GY0F[:, :C],
                                        in1=TY1[:, :C],
                                        op=mybir.AluOpType.mult)
                nc.gpsimd.tensor_tensor(out=GY1s[ci][:, :C], in0=GY1F[:, :C],
                                        in1=TY1[:, :C],
                                        op=mybir.AluOpType.mult)
                R = ftmp.tile([P, CA], F32, name="R")
                nc.vector.tensor_tensor(out=R[:, :C], in0=V[:, :C],
                                        in1=TX1[:, :C],
                                        op=mybir.AluOpType.mult)
                nc.vector.tensor_tensor(out=A0s[ci][:, :C], in0=R[:, :C],
                                        in1=GX0[:, :C],
                                        op=mybir.AluOpType.mult)
                nc.vector.tensor_tensor(out=A1s[ci][:, :C], in0=R[:, :C],
                                        in1=GX1[:, :C],
                                        op=mybir.AluOpType.mult)
                # cxp1 = xb - (511 + 64 pid);  ryc = yb - (511 + 32*band)
                nc.vector.tensor_scalar(out=CXP1s[ci][:, :C], in0=XBF[:],
                                        scalar1=SCX[:, 0:1], scalar2=None,
                                        op0=mybir.AluOpType.subtract)
                nc.vector.tensor_tensor(out=RYCs[ci][:, :C], in0=YBF[:],
                                        in1=RB[:, sl],
                                        op=mybir.AluOpType.subtract)

            # ---- phase B ----
            # band w accumulates at PSUM partitions 32*(w%2)+[0,32),
            # cols 64*(w//2)+[0,64)  (PE requires out base partition 0/32/64)
            PS = psp.tile([P, 512], F32)
            band_batches = []
            for band in range(NBANDS):
                j = band * nbb
                end = (band + 1) * nbb
                while j < end:
                    n = min(nbatch, end - j)
                    n = min(n, CA - (j % CA) if CA - (j % CA) > 0 else n)
                    if n % 2 and j + n < end:
                        n -= 1          # keep batches even for pairing
                    band_batches.append((band, j, n))
                    j += n

            def pap(tile_ap, off, dims):
                return bass.AP(tile_ap.tensor, tile_ap.offset + off, dims)

            for band, j0, nbt in band_batches:
                ci, jl = j0 // CA, j0 % CA
                npair = (nbt + 1) // 2
                # paired views: element (q, f, i) = block 2q+i, window pos f
                XC = bat.tile([P, NBATCH * 66], F16, name="XC")
                pdim = XC[:].ap[0]
                nc.vector.tensor_tensor(
                    out=pap(XC[:], 0, [pdim, [132, npair], [2, 66], [1, 2]]),
                    in0=pap(XIOTA[:], 0, [XIOTA[:].ap[0], [0, npair], [2, 66], [1, 2]]),
                    in1=pap(CXP1s[ci][:], jl, [CXP1s[ci][:].ap[0], [2, npair], [0, 66], [1, 2]]),
                    op=mybir.AluOpType.is_equal)
                YC = bat.tile([P, NBATCH * 34], F16, name="YC")
                nc.vector.tensor_tensor(
                    out=pap(YC[:], 0, [YC[:].ap[0], [68, npair], [2, 34], [1, 2]]),
                    in0=pap(YIOTA[:], 0, [YIOTA[:].ap[0], [0, npair], [2, 34], [1, 2]]),
                    in1=pap(RYCs[ci][:], jl, [RYCs[ci][:].ap[0], [2, npair], [0, 34], [1, 2]]),
                    op=mybir.AluOpType.is_equal)
                T0 = bat.tile([P, NBATCH * 34], F16, name="T0")
                nc.vector.tensor_tensor(
                    out=pap(T0[:], 0, [T0[:].ap[0], [68, npair], [2, 34], [1, 2]]),
                    in0=pap(YC[:], 0, [YC[:].ap[0], [68, npair], [2, 34], [1, 2]]),
                    in1=pap(GY0s[ci][:], jl, [GY0s[ci][:].ap[0], [2, npair], [0, 34], [1, 2]]),
                    op=mybir.AluOpType.mult)
                T1 = bat.tile([P, NBATCH * 34], F16, name="T1")
                nc.gpsimd.tensor_tensor(
                    out=pap(T1[:], 0, [T1[:].ap[0], [68, npair], [2, 34], [1, 2]]),
                    in0=pap(YC[:], 0, [YC[:].ap[0], [68, npair], [2, 34], [1, 2]]),
                    in1=pap(GY1s[ci][:], jl, [GY1s[ci][:].ap[0], [2, npair], [0, 34], [1, 2]]),
                    op=mybir.AluOpType.mult)
                # L[k, (q,p,i)] = gy0*(p==ry0) + gy1*(p==ry0+1)
                L = bat.tile([P, NBATCH * 32], F16, name="L")
                nc.vector.tensor_tensor(
                    out=pap(L[:], 0, [L[:].ap[0], [64, npair], [2, 32], [1, 2]]),
                    in0=pap(T0[:], 2, [T0[:].ap[0], [68, npair], [2, 32], [1, 2]]),
                    in1=pap(T1[:], 0, [T1[:].ap[0], [68, npair], [2, 32], [1, 2]]),
                    op=mybir.AluOpType.add)
                LA0 = bat.tile([P, NBATCH * 32], F16, name="LA0")
                nc.vector.tensor_tensor(
                    out=pap(LA0[:], 0, [LA0[:].ap[0], [64, npair], [2, 32], [1, 2]]),
                    in0=pap(L[:], 0, [L[:].ap[0], [64, npair], [2, 32], [1, 2]]),
                    in1=pap(A0s[ci][:], jl, [A0s[ci][:].ap[0], [2, npair], [0, 32], [1, 2]]),
                    op=mybir.AluOpType.mult)
                LA1 = bat.tile([P, NBATCH * 32], F16, name="LA1")
                nc.vector.tensor_tensor(
                    out=pap(LA1[:], 0, [LA1[:].ap[0], [64, npair], [2, 32], [1, 2]]),
                    in0=pap(L[:], 0, [L[:].ap[0], [64, npair], [2, 32], [1, 2]]),
                    in1=pap(A1s[ci][:], jl, [A1s[ci][:].ap[0], [2, npair], [0, 32], [1, 2]]),
                    op=mybir.AluOpType.mult)

                prow = (band % 2) * BAND_H
                pcol = (band // 2) * STRIP_W
                for b in range(nbt):
                    q, i = b // 2, b % 2
                    first = b + j0 == band * nbb
                    out_ap = PS[prow:prow + BAND_H, pcol:pcol + STRIP_W]
                    lhsT0 = pap(LA0[:], q * 64 + i, [LA0[:].ap[0], [2, 32]])
                    lhsT1 = pap(LA1[:], q * 64 + i, [LA1[:].ap[0], [2, 32]])
                    rhs0 = pap(XC[:], q * 132 + i + 2, [XC[:].ap[0], [2, 64]])
                    rhs1 = pap(XC[:], q * 132 + i, [XC[:].ap[0], [2, 64]])
                    nc.tensor.matmul(out=out_ap, lhsT=lhsT0, rhs=rhs0,
                                     start=first, stop=False)
                    nc.tensor.matmul(out=out_ap, lhsT=lhsT1, rhs=rhs1,
                                     start=False,
                                     stop=(j0 + b) == (band + 1) * nbb - 1)

            # ---- writeback ----
            OUT = pers.tile([P, 512], F32)
            nc.vector.tensor_copy(OUT[0:64, :], PS[0:64, :])
            for g in range(8):
                nc.sync.dma_start(strip_d[64 * g:64 * (g + 1), :],
                                  OUT[0:64, 64 * g:64 * (g + 1)])

    _split_multiwait(nc)
    return nc


def _shard(x, y, v):
    """Host sharding: assign each point (+boundary duplicates) to
    (core, band) buckets; return per-core padded [P, NB] arrays and nbb."""
    xp = (x + np.float32(1.0)) * np.float32(512.0)
    yp = (y + np.float32(1.0)) * np.float32(512.0)
    xb = np.floor(xp).astype(np.int32)
    yb = np.floor(yp).astype(np.int32)
    cx = xb - 512          # 0..511
    cy = yb - 512
    core = np.clip(cx >> 6, 0, NCORES - 1)   # 0..7
    band = np.clip(cy >> 5, 0, NBANDS - 1)   # 0..15
    xdup = (cx & 63) == 63
    xdup &= cx != 511      # col 1024 is clipped, no duplicate
    ydup = (cy & 31) == 31
    ydup &= cy != 511

    idx = np.arange(x.shape[0], dtype=np.int64)
    parts = [
        (idx, core, band),
        (idx[xdup], core[xdup] + 1, band[xdup]),
        (idx[ydup], core[ydup], band[ydup] + 1),
    ]
    bothdup = xdup & ydup
    parts.append((idx[bothdup], core[bothdup] + 1, band[bothdup] + 1))

    all_idx = np.concatenate([p[0] for p in parts])
    all_core = np.concatenate([p[1] for p in parts])
    all_band = np.concatenate([p[2] for p in parts])

    key = all_core * NBANDS + all_band
    order = np.argsort(key, kind="stable")
    all_idx = all_idx[order]
    key = key[order]
    counts = np.bincount(key, minlength=NCORES * NBANDS)
    maxc = int(counts.max())
    nbb = -(-maxc // P)                   # blocks per band
    nbb += nbb % 2                        # even, for block pairing
    NB = NBANDS * nbb
    slot = NB * P

    starts = np.zeros(NCORES * NBANDS + 1, dtype=np.int64)
    np.cumsum(counts, out=starts[1:])

    per_core = []
    for c in range(NCORES):
        xs = np.full(slot, 0.25, dtype=np.float32)
        ys = np.full(slot, 0.25, dtype=np.float32)
        vs = np.zeros(slot, dtype=np.float32)
        for w in range(NBANDS):
            k = c * NBANDS + w
            seg = all_idx[starts[k]:starts[k + 1]]
            off = w * nbb * P
            xs[off:off + seg.size] = x[seg]
            ys[off:off + seg.size] = y[seg]
            vs[off:off + seg.size] = v[seg]
        per_core.append({
            "xs": np.ascontiguousarray(xs.reshape(NB, P).T),
            "ys": np.ascontiguousarray(ys.reshape(NB, P).T),
            "vs": np.ascontiguousarray(vs.reshape(NB, P).T),
        })
    return per_core, nbb


_CACHE = {}


def kernel(x, y, values):
    x = np.asarray(x, dtype=np.float32)
    y = np.asarray(y, dtype=np.float32)
    v = np.asarray(values, dtype=np.float32)

    per_core, nbb = _shard(x, y, v)
    if nbb not in _CACHE:
        _CACHE[nbb] = _build_module(nbb)
    nc = _CACHE[nbb]

    res = run_bass_kernel_spmd(nc, per_core, core_ids=list(range(NCORES)))

    img = np.zeros((1024, 1024), dtype=np.float32)
    for c in range(NCORES):
        img[512:1024, 512 + 64 * c:512 + 64 * (c + 1)] = res.results[c]["strip"]
    return img



# revision 9
# speedup vs baseline: 1.7100x; 1.7100x over previous
"""Gaussian 2x2 splat (DifferentiableSquareSensor) on 8 Trainium2 NeuronCores.

Full inputs in, full 1024x1024 image out.

Math: x,y are uniform in [0,1), so pixel coords land in [512,1024) and with
sigma=0.1 every Gaussian tap except the nearest 2x2 neighborhood is <= e^-50
(~2e-22 relative) -- invisible in fp32.  The splat therefore reduces to a
separable 2x2 deposit with weights  g(t)=exp(-50 t^2), g(1-t)  per axis,
normalized by (gx0+gx1)(gy0+gy1).

Distribution: points are sharded to cores by 64-column x-range of the active
512x512 region, and within a core are bucketed by (32-col strip, 16-row band)
with boundary duplication.  Each core computes its [512, 64] range on-device:
  phase A: bulk fp32 coordinate/weight math (ACT + DVE + Pool)
  phase B: per-128-point-block one-hot placement tiles built with
           broadcast-AP tensor ops (cross-bucket batched), then two PE
           matmuls per block accumulate the 2x2 outer products into PSUM
           [16, 32] per-bucket accumulators.
The host only shards/buckets/pads inputs and reassembles the strips.
"""

import json
import os
import sys

import numpy as np

for _p in ("/opt/trn_rl_repo", "/root/.axon_site/_ro/trn_rl_repo"):
    if os.path.isdir(_p) and _p not in sys.path:
        sys.path.append(_p)

import concourse.bass as bass
import concourse.mybir as mybir
from concourse.bass_utils import run_bass_kernel_spmd
from concourse.tile import TileContext

P = 128
NCORES = 8
SW = 32               # strip width (cols per bucket)
BH = 16               # band height (rows per bucket)
NSTRIP = 64 // SW     # strips per core (2)
NBAND = 512 // BH     # bands per core (32)
NBUCKET = NSTRIP * NBAND          # 64, bucket = s*NBAND + w
XWIN = SW + 2         # 34
YWIN = BH + 2         # 18
NBATCH = 128          # blocks per batched phase-B build group
CA = 512              # phase-A chunk columns (blocks)
F32 = mybir.dt.float32
F16 = mybir.dt.float16


def _split_multiwait(nc):
    """This walrus build rejects >1 sync-wait per instruction; split extras
    into single-wait NoOps placed immediately before on the same engine."""
    orig = nc.to_json_bytes

    def patched():
        js = json.loads(orig().decode())
        for fn in js["functions"]:
            for blk in fn["blocks"]:
                newlist = []
                for inst in blk["instructions"]:
                    si = inst.get("sync_info")
                    ow = (si or {}).get("on_wait") or []
                    if len(ow) > 1:
                        for k, w in enumerate(ow[:-1]):
                            newlist.append({
                                "name": f"{inst['name']}-w{k}",
                                "opcode": "NoOp",
                                "engine": inst["engine"],
                                "ins": [], "outs": [],
                                "sync_info": {"on_wait": [w], "on_update": []},
                                "bass_nofuse": True,
                            })
                        si["on_wait"] = [ow[-1]]
                    newlist.append(inst)
                blk["instructions"] = newlist
        return json.dumps(js).encode()

    nc.to_json_bytes = patched


def _build_module(nbb):
    """Build the SPMD bass module for per-bucket block count nbb (even).
    NB = NBUCKET*nbb total blocks, bucket-major (strip s outer, band w
    inner), each block = 128 points on partitions."""
    NB = NBUCKET * nbb
    HALF = NB // 2        # columns of strip 0 vs strip 1
    nc = bass.Bass("TRN2", target_bir_lowering=False, debug=False,
                   num_devices=NCORES)
    xs_d = nc.dram_tensor("xs", [P, NB], F32, kind="ExternalInput")
    ys_d = nc.dram_tensor("ys", [P, NB], F32, kind="ExternalInput")
    vs_d = nc.dram_tensor("vs", [P, NB], F32, kind="ExternalInput")
    strip_d = nc.dram_tensor("strip", [512, 64], F32, kind="ExternalOutput")

    nchunks = (NB + CA - 1) // CA
    AF = mybir.ActivationFunctionType
    TT = mybir.AluOpType

    with TileContext(nc) as tc:
        with (
            tc.tile_pool(name="persist", bufs=1) as pers,
            tc.tile_pool(name="chunk", bufs=2) as chk,
            tc.tile_pool(name="ftmp", bufs=1) as ftmp,
            tc.tile_pool(name="batch", bufs=2) as bat,
            tc.tile_pool(name="psum", bufs=1, space="PSUM") as psp,
        ):
            # ---- one-time constants ----
            PIDU = pers.tile([P, 1], mybir.dt.uint32)
            nc.gpsimd.dma_start(
                PIDU[:], nc.partition_id_tensor[0:1, 0:1].to_broadcast([P, 1]))
            PIDF = pers.tile([P, 1], F32)
            nc.vector.tensor_copy(PIDF[:], PIDU[:])
            # SCX[s] = 511 + 64*pid + SW*s  (cxp1 = xpix - SCX)
            SCX = [pers.tile([P, 1], F32, name=f"SCX{s}")
                   for s in range(NSTRIP)]
            for s in range(NSTRIP):
                nc.vector.tensor_scalar(out=SCX[s][:], in0=PIDF[:],
                                        scalar1=64.0,
                                        scalar2=511.0 + SW * s,
                                        op0=TT.mult, op1=TT.add)
            # RB[p, j] = 511 + BH*band(j)   (ryc = ypix_base - RB)
            RB = pers.tile([P, NB], F32)
            nc.gpsimd.iota(RB[:], pattern=[[0, NSTRIP], [BH, NBAND], [0, nbb]],
                           base=511, channel_multiplier=0,
                           allow_small_or_imprecise_dtypes=True)
            # pair-duplicated iotas: values 0,0,1,1,... so two blocks'
            # one-hots interleave in adjacent fp16 lanes (DVE 2x mode)
            XIOTA = pers.tile([P, 2 * XWIN], F16)
            nc.gpsimd.iota(XIOTA[:], pattern=[[1, XWIN], [0, 2]], base=0,
                           channel_multiplier=0,
                           allow_small_or_imprecise_dtypes=True)
            YIOTA = pers.tile([P, 2 * YWIN], F16)
            nc.gpsimd.iota(YIOTA[:], pattern=[[1, YWIN], [0, 2]], base=0,
                           channel_multiplier=0,
                           allow_small_or_imprecise_dtypes=True)

            # ---- per-point arrays, one tile per phase-A chunk so that
            # phase-B batches only depend on their own chunk (overlap) ----
            def chunk_tiles(nm):
                return [pers.tile([P, min(CA, NB - i * CA)], F16,
                                  name=f"{nm}{i}") for i in range(nchunks)]
            CXP1s = chunk_tiles("CXP1")
            RYCs = chunk_tiles("RYC")
            GY0s = chunk_tiles("GY0")
            GY1s = chunk_tiles("GY1")
            A0s = chunk_tiles("A0")
            A1s = chunk_tiles("A1")

            # ---- phase A ----
            for ci in range(nchunks):
                j0 = ci * CA
                C = min(CA, NB - j0)
                sl = slice(j0, j0 + C)
                X = chk.tile([P, CA], F32, name="X")
                Y = chk.tile([P, CA], F32, name="Y")
                V = chk.tile([P, CA], F32, name="V")
                nc.sync.dma_start(X[:, :C], xs_d[:, sl])
                nc.sync.dma_start(Y[:, :C], ys_d[:, sl])
                nc.sync.dma_start(V[:, :C], vs_d[:, sl])

                XP = ftmp.tile([P, CA], F32, name="XP")
                nc.scalar.activation(XP[:, :C], X[:, :C], AF.Copy,
                                     bias=512.0, scale=512.0)
                YP = ftmp.tile([P, CA], F32, name="YP")
                nc.scalar.activation(YP[:, :C], Y[:, :C], AF.Copy,
                                     bias=512.0, scale=512.0)

                # exact floor/frac: xp in [512,1024) has fp32 exponent 9, so
                # masking the low 14 mantissa bits IS floor(xp); frac exact.
                XB = ftmp.tile([P, CA], F32, name="XB")
                nc.vector.tensor_scalar(out=XB[:, :C].bitcast(mybir.dt.int32),
                                        in0=XP[:, :C].bitcast(mybir.dt.int32),
                                        scalar1=-16384, scalar2=None,
                                        op0=TT.bitwise_and)
                YB = ftmp.tile([P, CA], F32, name="YB")
                nc.vector.tensor_scalar(out=YB[:, :C].bitcast(mybir.dt.int32),
                                        in0=YP[:, :C].bitcast(mybir.dt.int32),
                                        scalar1=-16384, scalar2=None,
                                        op0=TT.bitwise_and)
                TX = ftmp.tile([P, CA], F32, name="TX")
                nc.vector.tensor_tensor(out=TX[:, :C], in0=XP[:, :C],
                                        in1=XB[:, :C], op=TT.subtract)
                TY = ftmp.tile([P, CA], F32, name="TY")
                nc.vector.tensor_tensor(out=TY[:, :C], in0=YP[:, :C],
                                        in1=YB[:, :C], op=TT.subtract)
                UX = ftmp.tile([P, CA], F32, name="UX")
                nc.vector.tensor_scalar(out=UX[:, :C], in0=TX[:, :C],
                                        scalar1=1.0, scalar2=None,
                                        op0=TT.subtract)
                UY = ftmp.tile([P, CA], F32, name="UY")
                nc.vector.tensor_scalar(out=UY[:, :C], in0=TY[:, :C],
                                        scalar1=1.0, scalar2=None,
                                        op0=TT.subtract)

                # gx0 = exp(-50 tx^2), gx1 = exp(-50 (tx-1)^2); same for y
                SX = ftmp.tile([P, CA], F32, name="SX")
                nc.scalar.activation(SX[:, :C], TX[:, :C], AF.Square)
                SUX = ftmp.tile([P, CA], F32, name="SUX")
                nc.scalar.activation(SUX[:, :C], UX[:, :C], AF.Square)
                SY = ftmp.tile([P, CA], F32, name="SY")
                nc.scalar.activation(SY[:, :C], TY[:, :C], AF.Square)
                SUY = ftmp.tile([P, CA], F32, name="SUY")
                nc.scalar.activation(SUY[:, :C], UY[:, :C], AF.Square)
                GX0 = ftmp.tile([P, CA], F32, name="GX0")
                nc.scalar.activation(GX0[:, :C], SX[:, :C], AF.Exp,
                                     bias=0.0, scale=-50.0)
                GX1 = ftmp.tile([P, CA], F32, name="GX1")
                nc.scalar.activation(GX1[:, :C], SUX[:, :C], AF.Exp,
                                     bias=0.0, scale=-50.0)
                GY0F = ftmp.tile([P, CA], F32, name="GY0F")
                nc.scalar.activation(GY0F[:, :C], SY[:, :C], AF.Exp,
                                     bias=0.0, scale=-50.0)
                GY1F = ftmp.tile([P, CA], F32, name="GY1F")
                nc.scalar.activation(GY1F[:, :C], SUY[:, :C], AF.Exp,
                                     bias=0.0, scale=-50.0)

                # R = v / ((gx0+gx1)(gy0+gy1))
                ZX = ftmp.tile([P, CA], F32, name="ZX")
                nc.vector.tensor_tensor(out=ZX[:, :C], in0=GX0[:, :C],
                                        in1=GX1[:, :C], op=TT.add)
                ZY = ftmp.tile([P, CA], F32, name="ZY")
                nc.vector.tensor_tensor(out=ZY[:, :C], in0=GY0F[:, :C],
                                        in1=GY1F[:, :C], op=TT.add)
                Z = ftmp.tile([P, CA], F32, name="Z")
                nc.vector.tensor_tensor(out=Z[:, :C], in0=ZX[:, :C],
                                        in1=ZY[:, :C], op=TT.mult)
                # 1/Z via exp(-ln Z) on ACT (same act table set; ~1e-4 rel)
                LNZ = ftmp.tile([P, CA], F32, name="LNZ")
                nc.scalar.activation(LNZ[:, :C], Z[:, :C], AF.Ln)
                RZ = ftmp.tile([P, CA], F32, name="RZ")
                nc.scalar.activation(RZ[:, :C], LNZ[:, :C], AF.Exp,
                                     bias=0.0, scale=-1.0)
                # keep every fp16-stored factor bounded: gy' = gy/Zy <= 1,
                # a' = v*gx/Zx <= |v|  (their product is the exact deposit)
                TY1 = ftmp.tile([P, CA], F32, name="TY1")
                nc.vector.tensor_tensor(out=TY1[:, :C], in0=ZX[:, :C],
                                        in1=RZ[:, :C], op=TT.mult)
                TX1 = ftmp.tile([P, CA], F32, name="TX1")
                nc.gpsimd.tensor_tensor(out=TX1[:, :C], in0=ZY[:, :C],
                                        in1=RZ[:, :C], op=TT.mult)
                nc.vector.tensor_tensor(out=GY0s[ci][:, :C], in0=GY0F[:, :C],
                                        in1=TY1[:, :C], op=TT.mult)
                nc.gpsimd.tensor_tensor(out=GY1s[ci][:, :C], in0=GY1F[:, :C],
                                        in1=TY1[:, :C], op=TT.mult)
                R = ftmp.tile([P, CA], F32, name="R")
                nc.vector.tensor_tensor(out=R[:, :C], in0=V[:, :C],
                                        in1=TX1[:, :C], op=TT.mult)
                nc.vector.tensor_tensor(out=A0s[ci][:, :C], in0=R[:, :C],
                                        in1=GX0[:, :C], op=TT.mult)
                nc.vector.tensor_tensor(out=A1s[ci][:, :C], in0=R[:, :C],
                                        in1=GX1[:, :C], op=TT.mult)
                # cxp1 = xpix_base - (511 + 64 pid + SW s);  split at strip
                # boundary column HALF so the subtrahend is per-partition
                for (lo, hi, s) in ((j0, min(j0 + C, HALF), 0),
                                    (max(j0, HALF), j0 + C, 1)):
                    if lo < hi:
                        a, b = lo - j0, hi - j0
                        nc.vector.tensor_scalar(out=CXP1s[ci][:, a:b],
                                                in0=XB[:, a:b],
                                                scalar1=SCX[s][:, 0:1],
                                                scalar2=None, op0=TT.subtract)
                # ryc = ypix_base - (511 + BH band)
                nc.vector.tensor_tensor(out=RYCs[ci][:, :C], in0=YB[:, :C],
                                        in1=RB[:, sl], op=TT.subtract)

            # ---- phase B ----
            # bucket b (= s*NBAND + w) accumulates at PSUM partitions
            # 32*(w%2)+[0,BH), cols 32*(w//2) + 512*s + [0,SW)
            PS = psp.tile([P, 1024], F32)
            batches = []
            for ci in range(nchunks):
                lo = ci * CA
                hi = min(lo + CA, NB)
                j = lo
                while j < hi:
                    n = min(NBATCH, hi - j)
                    batches.append((ci, j, n))
                    j += n

            def pap(tile_ap, off, dims):
                return bass.AP(tile_ap.tensor, tile_ap.offset + off, dims)

            for ci, j0, nbt in batches:
                jl = j0 - ci * CA
                npair = (nbt + 1) // 2
                # paired views: element (q, f, i) = block 2q+i, window pos f
                XC = bat.tile([P, NBATCH * XWIN], F16, name="XC")
                pdim = XC[:].ap[0]
                nc.vector.tensor_tensor(
                    out=pap(XC[:], 0, [pdim, [2 * XWIN, npair], [2, XWIN], [1, 2]]),
                    in0=pap(XIOTA[:], 0, [XIOTA[:].ap[0], [0, npair], [2, XWIN], [1, 2]]),
                    in1=pap(CXP1s[ci][:], jl, [CXP1s[ci][:].ap[0], [2, npair], [0, XWIN], [1, 2]]),
                    op=TT.is_equal)
                YC = bat.tile([P, NBATCH * YWIN], F16, name="YC")
                nc.vector.tensor_tensor(
                    out=pap(YC[:], 0, [YC[:].ap[0], [2 * YWIN, npair], [2, YWIN], [1, 2]]),
                    in0=pap(YIOTA[:], 0, [YIOTA[:].ap[0], [0, npair], [2, YWIN], [1, 2]]),
                    in1=pap(RYCs[ci][:], jl, [RYCs[ci][:].ap[0], [2, npair], [0, YWIN], [1, 2]]),
                    op=TT.is_equal)
                T0 = bat.tile([P, NBATCH * YWIN], F16, name="T0")
                nc.vector.tensor_tensor(
                    out=pap(T0[:], 0, [T0[:].ap[0], [2 * YWIN, npair], [2, YWIN], [1, 2]]),
                    in0=pap(YC[:], 0, [YC[:].ap[0], [2 * YWIN, npair], [2, YWIN], [1, 2]]),
                    in1=pap(GY0s[ci][:], jl, [GY0s[ci][:].ap[0], [2, npair], [0, YWIN], [1, 2]]),
                    op=TT.mult)
                T1 = bat.tile([P, NBATCH * YWIN], F16, name="T1")
                nc.gpsimd.tensor_tensor(
                    out=pap(T1[:], 0, [T1[:].ap[0], [2 * YWIN, npair], [2, YWIN], [1, 2]]),
                    in0=pap(YC[:], 0, [YC[:].ap[0], [2 * YWIN, npair], [2, YWIN], [1, 2]]),
                    in1=pap(GY1s[ci][:], jl, [GY1s[ci][:].ap[0], [2, npair], [0, YWIN], [1, 2]]),
                    op=TT.mult)
                # L[k, (q,r,i)] = gy0*(r+1==ryc) + gy1*(r==ryc)
                L = bat.tile([P, NBATCH * BH], F16, name="L")
                nc.vector.tensor_tensor(
                    out=pap(L[:], 0, [L[:].ap[0], [2 * BH, npair], [2, BH], [1, 2]]),
                    in0=pap(T0[:], 2, [T0[:].ap[0], [2 * YWIN, npair], [2, BH], [1, 2]]),
                    in1=pap(T1[:], 0, [T1[:].ap[0], [2 * YWIN, npair], [2, BH], [1, 2]]),
                    op=TT.add)
                LA0 = bat.tile([P, NBATCH * BH], F16, name="LA0")
                nc.vector.tensor_tensor(
                    out=pap(LA0[:], 0, [LA0[:].ap[0], [2 * BH, npair], [2, BH], [1, 2]]),
                    in0=pap(L[:], 0, [L[:].ap[0], [2 * BH, npair], [2, BH], [1, 2]]),
                    in1=pap(A0s[ci][:], jl, [A0s[ci][:].ap[0], [2, npair], [0, BH], [1, 2]]),
                    op=TT.mult)
                LA1 = bat.tile([P, NBATCH * BH], F16, name="LA1")
                nc.vector.tensor_tensor(
                    out=pap(LA1[:], 0, [LA1[:].ap[0], [2 * BH, npair], [2, BH], [1, 2]]),
                    in0=pap(L[:], 0, [L[:].ap[0], [2 * BH, npair], [2, BH], [1, 2]]),
                    in1=pap(A1s[ci][:], jl, [A1s[ci][:].ap[0], [2, npair], [0, BH], [1, 2]]),
                    op=TT.mult)

                for b in range(nbt):
                    q, i = b // 2, b % 2
                    g = j0 + b          # global block
                    bkt = g // nbb
                    s, w = bkt // NBAND, bkt % NBAND
                    prow = 32 * (w % 2)
                    pcol = 32 * (w // 2) + 512 * s
                    first = (g % nbb) == 0
                    last = (g % nbb) == nbb - 1
                    out_ap = PS[prow:prow + BH, pcol:pcol + SW]
                    lhsT0 = pap(LA0[:], q * 2 * BH + i, [LA0[:].ap[0], [2, BH]])
                    lhsT1 = pap(LA1[:], q * 2 * BH + i, [LA1[:].ap[0], [2, BH]])
                    rhs0 = pap(XC[:], q * 2 * XWIN + i + 2, [XC[:].ap[0], [2, SW]])
                    rhs1 = pap(XC[:], q * 2 * XWIN + i, [XC[:].ap[0], [2, SW]])
                    nc.tensor.matmul(out=out_ap, lhsT=lhsT0, rhs=rhs0,
                                     start=first, stop=False)
                    nc.tensor.matmul(out=out_ap, lhsT=lhsT1, rhs=rhs1,
                                     start=False, stop=last)

            # ---- writeback ----
            # PS[32(w%2)+r, 32(w//2)+512 s+c] -> strip[BH w + r, SW s + c]
            OUT = pers.tile([P, 1024], F32)
            nc.vector.tensor_copy(OUT[0:48, :], PS[0:48, :])
            full = strip_d[0:512, 0:64]
            for k in range(2):
                for s in range(NSTRIP):
                    band_rows = OUT[32 * k:32 * k + BH, :]
                    src = pap(band_rows, 512 * s,
                              [band_rows.ap[0], [32, NBAND // 2], [1, SW]])
                    dst = bass.AP(full.tensor,
                                  full.offset + (BH * k) * 64 + SW * s,
                                  [[64, BH], [2 * BH * 64, NBAND // 2], [1, SW]])
                    nc.sync.dma_start(dst, src)

    _split_multiwait(nc)
    return nc


def _shard(x, y, v):
    """Host sharding: assign each point (+boundary duplicates) to
    (core, bucket) with bucket = strip*NBAND + band; return per-core padded
    [P, NB] arrays and nbb."""
    xp = (x + np.float32(1.0)) * np.float32(512.0)
    yp = (y + np.float32(1.0)) * np.float32(512.0)
    cx = np.floor(xp).astype(np.int32) - 512          # 0..511
    cy = np.floor(yp).astype(np.int32) - 512

    def assign(cx, cy):
        core = np.clip(cx >> 6, 0, NCORES - 1)
        strip = (np.clip(cx, 0, 511) >> 5) & (NSTRIP - 1)
        band = np.clip(cy >> 4, 0, NBAND - 1)
        return core * NBUCKET + strip * NBAND + band

    xdup = ((cx & (SW - 1)) == SW - 1) & (cx != 511)
    ydup = ((cy & (BH - 1)) == BH - 1) & (cy != 511)
    bothdup = xdup & ydup

    idx = np.arange(x.shape[0], dtype=np.int64)
    parts = [
        (idx, assign(cx, cy)),
        (idx[xdup], assign(cx[xdup] + 1, cy[xdup])),
        (idx[ydup], assign(cx[ydup], cy[ydup] + 1)),
        (idx[bothdup], assign(cx[bothdup] + 1, cy[bothdup] + 1)),
    ]
    all_idx = np.concatenate([p[0] for p in parts])
    key = np.concatenate([p[1] for p in parts])

    order = np.argsort(key, kind="stable")
    all_idx = all_idx[order]
    key = key[order]
    counts = np.bincount(key, minlength=NCORES * NBUCKET)
    maxc = int(counts.max())
    nbb = -(-maxc // P)                   # blocks per bucket
    nbb += nbb % 2                        # even, for block pairing
    NB = NBUCKET * nbb
    slot = NB * P

    starts = np.zeros(NCORES * NBUCKET + 1, dtype=np.int64)
    np.cumsum(counts, out=starts[1:])

    per_core = []
    for c in range(NCORES):
        xs = np.full(slot, 0.25, dtype=np.float32)
        ys = np.full(slot, 0.25, dtype=np.float32)
        vs = np.zeros(slot, dtype=np.float32)
        for bkt in range(NBUCKET):
            k = c * NBUCKET + bkt
            seg = all_idx[starts[k]:starts[k + 1]]
            off = bkt * nbb * P
            xs[off:off + seg.size] = x[seg]
            ys[off:off + seg.size] = y[seg]
            vs[off:off + seg.size] = v[seg]
        per_core.append({
            "xs": np.ascontiguousarray(xs.reshape(NB, P).T),
            "ys": np.ascontiguousarray(ys.reshape(NB, P).T),
            "vs": np.ascontiguousarray(vs.reshape(NB, P).T),
        })
    return per_core, nbb


_CACHE = {}


def kernel(x, y, values):
    x = np.asarray(x, dtype=np.float32)
    y = np.asarray(y, dtype=np.float32)
    v = np.asarray(values, dtype=np.float32)

    per_core, nbb = _shard(x, y, v)
    if nbb not in _CACHE:
        _CACHE[nbb] = _build_module(nbb)
    nc = _CACHE[nbb]

    res = run_bass_kernel_spmd(nc, per_core, core_ids=list(range(NCORES)))

    img = np.zeros((1024, 1024), dtype=np.float32)
    for c in range(NCORES):
        img[512:1024, 512 + 64 * c:512 + 64 * (c + 1)] = res.results[c]["strip"]
    return img
